# revision 12
# baseline (speedup 1.0000x reference)
"""Causal self-attention Trainium2 kernel (B=2, T=4096, C=768, H=12, D=64).

Sharding: 8 cores = 2 batches x 4 head-groups (3 heads each).
Each core computes, for its (batch b, heads h0..h2):
  - QKV projection from x[b].T (transposed + bf16-cast on host)
  - causal flash attention in score-transposed layout (S^T tiles [k=128, q=512])
  - output projection partial out_p = sum_h (O_h / l_h) @ Wout[h*64:(h+1)*64]
Host gathers: out[b] = sum of the 4 partials + bout.

v2: all matmul operands bf16 (fp32 PSUM accumulate); causal diagonal
supertiles are column-sliced so fully-masked q-subtiles are never computed,
exp'd, or PV'd; masking reduces to one [128,128] triangular block per
diagonal tile.
"""

import numpy as np
from contextlib import ExitStack

import concourse.bass as bass
import concourse.bacc as bacc
import concourse.mybir as mybir
import concourse.tile as tile
from concourse.bass_utils import run_bass_kernel_spmd

B, T, C, H, D = 2, 4096, 768, 12, 64
NCORES = 8
HPC = 3  # heads per core
GPB = 4  # head-groups per batch
SCALE = float(np.sqrt(D))  # 8.0
QS = 512  # q supertile (columns of S^T tiles)
KT = 128  # k tile (partitions of S^T tiles)
NQS = T // QS  # 8
NCH = C // 128  # 6 contraction chunks
G = 2  # S^T tiles per exp batch
VW = HPC * 64 + HPC  # vo block: [V0|1|V1|1|V2|1] = 195 cols

F32 = mybir.dt.float32
BF16 = mybir.dt.bfloat16
AX = mybir.AxisListType
ALU = mybir.AluOpType
ACTF = mybir.ActivationFunctionType


def build_nc(with_qkv_bias: bool, repeat: int = 1, parts=('qkv', 'attn', 'proj')):
    nc = bacc.Bacc()

    xt = nc.dram_tensor("xt", [128, NCH, T], BF16, kind="ExternalInput")
    wqk = nc.dram_tensor("wqk", [128, NCH * 384], BF16, kind="ExternalInput")
    wqk_b = nc.dram_tensor("wqk_b", [1, 384], BF16, kind="ExternalInput")
    wv = nc.dram_tensor("wv", [128, NCH * 256], BF16, kind="ExternalInput")
    wv_b = nc.dram_tensor("wv_b", [1, 256], BF16, kind="ExternalInput")
    wout = nc.dram_tensor("wout", [HPC, 64, C], BF16, kind="ExternalInput")
    trid = nc.dram_tensor("trid", [128, 128], BF16, kind="ExternalInput")
    ones_d = nc.dram_tensor("ones_d", [1, QS], BF16, kind="ExternalInput")
    out_p = nc.dram_tensor("out_p", [T, C], BF16, kind="ExternalOutput")

    do_qkv = 'qkv' in parts
    do_attn = 'attn' in parts
    do_proj = 'proj' in parts
    k_exp = 'noexp' not in parts
    k_mask = 'nomask' not in parts
    k_pv = 'nopv' not in parts
    k_norm = 'nonorm' not in parts

    with tile.TileContext(nc) as tc, ExitStack() as ctx:
        rep_scope = ExitStack()
        if repeat > 1:
            rep_scope.enter_context(tc.For_i(0, repeat, 1))
        const = ctx.enter_context(tc.tile_pool(name="const", bufs=1))

        wqk_sb = const.tile([128, NCH * 384], BF16, tag="wqk")
        nc.sync.dma_start(wqk_sb[:], wqk[:])
        wv_sb = const.tile([128, NCH * 256], BF16, tag="wv")
        nc.sync.dma_start(wv_sb[:], wv[:])
        wvb_sb = const.tile([1, 256], BF16, tag="wvb")
        # wv_b always carries the ones column for the softmax denominator
        nc.sync.dma_start(wvb_sb[:], wv_b[:])
        wqkb_sb = const.tile([1, 384], BF16, tag="wqkb")
        if with_qkv_bias:
            nc.sync.dma_start(wqkb_sb[:], wqk_b[:])
        tri_sb = const.tile([128, 128], BF16, tag="tri")
        nc.sync.dma_start(tri_sb[:], trid[:])
        wout_sb = []
        for h in range(HPC):
            wsb = const.tile([64, C], BF16, tag=f"wout{h}", name=f"wout_sb{h}")
            nc.sync.dma_start(wsb[:], wout[h])
            wout_sb.append(wsb)

        ones_row = const.tile([1, QS], BF16, tag="ones_row")
        nc.sync.dma_start(ones_row[:], ones_d[:])
        ones65 = const.tile([128, 65], BF16, tag="ones65")
        nc.sync.dma_start(ones65[64:65, :], ones_d[:, 0:65])

        qt01 = const.tile([128, T], BF16, tag="qt01")
        kt01 = const.tile([128, T], BF16, tag="kt01")
        qkt2 = const.tile([128, T], BF16, tag="qkt2")
        dup2 = const.tile([128, T], BF16, tag="dup2")
        vo = const.tile([128, (T // 128) * VW], BF16, tag="vo")
        ot = [const.tile([65, T], BF16, tag=f"ot{h}", name=f"ot{h}") for h in range(HPC)]

        main_scope = ExitStack()
        xpool = main_scope.enter_context(tc.tile_pool(name="xt", bufs=3))
        qkv_ps = main_scope.enter_context(tc.tile_pool(name="qkvps", bufs=2, space="PSUM"))
        sg_ps = main_scope.enter_context(tc.tile_pool(name="sgps", bufs=2, space="PSUM"))
        ot_ps = main_scope.enter_context(tc.tile_pool(name="otps", bufs=2, space="PSUM"))
        ppool = main_scope.enter_context(tc.tile_pool(name="pt", bufs=4))

        def qkv_units(it):
            """Emit x DMAs now; return PE work-unit closures to interleave."""
            tw = slice(it * QS, (it + 1) * QS)
            xtile = xpool.tile([128, NCH, QS], BF16, tag="x", name=f"xt{it}")
            nc.sync.dma_start(xtile[:], xt[:, :, tw])
            xts = [xtile[:, c, :] for c in range(NCH)]
            units = []

            def pack_unit(p, dest):
                def emit():
                    ps = qkv_ps.tile([128, QS], F32, tag="qkv", name=f"qk{it}_{p}")
                    for c in range(NCH):
                        nc.tensor.matmul(
                            ps[:],
                            wqk_sb[:, c * 384 + p * 128: c * 384 + (p + 1) * 128],
                            xts[c][:],
                            start=(c == 0),
                            stop=(not with_qkv_bias and c == NCH - 1),
                        )
                    if with_qkv_bias:
                        nc.tensor.matmul(
                            ps[:], wqkb_sb[:, p * 128:(p + 1) * 128], ones_row[:],
                            start=False, stop=True,
                        )
                    with nc.allow_low_precision(reason="bf16 matmul operand"):
                        nc.vector.tensor_copy(dest[:, tw], ps[:])
                    if p == 2:
                        nc.sync.dma_start(dup2[0:64, tw], qkt2[64:128, tw])
                        nc.sync.dma_start(dup2[64:128, tw], qkt2[0:64, tw])
                return emit

            def v_unit(st):
                def emit():
                    tcn = it * (QS // 128) + st
                    ps = qkv_ps.tile([128, 256], F32, tag="qkv", name=f"v{it}_{st}")
                    for c in range(NCH):
                        nc.tensor.matmul(
                            ps[:],
                            xts[c][:, st * 128:(st + 1) * 128],
                            wv_sb[:, c * 256:(c + 1) * 256],
                            start=(c == 0),
                            stop=False,
                        )
                    # always: injects the ones column (softmax denominator)
                    nc.tensor.matmul(
                        ps[:], ones_row[:, st * 128:(st + 1) * 128], wvb_sb[:],
                        start=False, stop=True,
                    )
                    with nc.allow_low_precision(reason="bf16 matmul operand"):
                        nc.vector.tensor_copy(
                            vo[:, tcn * VW:(tcn + 1) * VW], ps[:, 0:VW]
                        )
                return emit

            for p, dest in ((0, qt01), (1, kt01), (2, qkt2)):
                units.append(pack_unit(p, dest))
            for st in range(QS // 128):
                units.append(v_unit(st))
            return units

        def attend(s, jobs, work_q=None):
            """jobs: list of (h, ktb_fn, qtb_fn)."""
            qw = slice(s * QS, (s + 1) * QS)
            nk = 4 * (s + 1)
            otps = {}
            for h, _, _ in jobs:
                otps[h] = ot_ps.tile([65, QS], F32, tag="ot", name=f"otp{h}_{s}")

            def mstart(jj):  # first unmasked col (within the 512-q window)
                m = jj - (nk - 4)
                return m * 128 if m > 0 else 0

            def emit_pv(pts, g0, gn):
                for h, _, _ in jobs:
                    for jl in range(gn):
                        jj = g0 + jl
                        c0 = mstart(jj)
                        m = jj - (nk - 4)
                        if k_mask and m >= 0:
                            # triangle block only: q-subtile == k-tile index
                            blk = slice(jl * QS + c0, jl * QS + c0 + 128)
                            nc.vector.tensor_tensor(
                                pts[h][:, blk], pts[h][:, blk], tri_sb[:],
                                op=ALU.mult,
                            )
                        if k_pv:
                            nc.tensor.matmul(
                                otps[h][:, c0:QS],
                                vo[:, jj * VW + h * 65: jj * VW + (h + 1) * 65],
                                pts[h][:, jl * QS + c0:(jl + 1) * QS],
                                start=(jj == 0),
                                stop=(jj == nk - 1),
                                skip_group_check=True,
                            )

            prev = None
            for g0 in range(0, nk, G):
                gn = min(G, nk - g0)
                sgps, pts = {}, {}
                for h, ktb_fn, qtb_fn in jobs:
                    sgps[h] = sg_ps.tile([128, G * QS], F32, tag="sg",
                                         name=f"sg{h}_{s}_{g0}")
                for jl in range(gn):
                    jj = g0 + jl
                    c0 = mstart(jj)
                    for h, ktb_fn, qtb_fn in jobs:
                        nc.tensor.matmul(
                            sgps[h][:, jl * QS + c0:(jl + 1) * QS],
                            ktb_fn(jj),
                            qtb_fn(jj)[:, s * QS + c0:(s + 1) * QS],
                            start=True, stop=True,
                        )
                # PV for the previous group fills PE while ACT exps this group
                if prev is not None:
                    emit_pv(*prev)
                if work_q:
                    work_q.pop(0)()
                for h, _, _ in jobs:
                    pt = ppool.tile([128, G * QS], BF16, tag="pt",
                                    name=f"pt{h}_{s}_{g0}")
                    if k_exp:
                        # fuse contiguous full tiles into one ACT instr
                        runs = []
                        for jl in range(gn):
                            c0 = mstart(g0 + jl)
                            lo, hi = jl * QS + c0, (jl + 1) * QS
                            if runs and runs[-1][1] == lo and c0 == 0:
                                runs[-1][1] = hi
                            else:
                                runs.append([lo, hi])
                        for lo, hi in runs:
                            nc.scalar.activation(
                                pt[:, lo:hi], sgps[h][:, lo:hi], ACTF.Exp,
                                scale=1.0 / SCALE,
                            )
                    pts[h] = pt
                prev = (pts, g0, gn)
            if prev is not None:
                emit_pv(*prev)
            for h, _, _ in jobs:
                if not k_pv:
                    continue
                with nc.allow_low_precision(reason="bf16 matmul operand"):
                    nc.vector.tensor_copy(ot[h][:, qw], otps[h][:])

        heads = (
            (kt01[0:64, :], qt01[0:64, :]),
            (kt01[64:128, :], qt01[64:128, :]),
        )

        def attend_all(s, work_q):
            attend(s, [
                (0, lambda jj: heads[0][0][:, jj * KT:(jj + 1) * KT],
                    lambda jj: heads[0][1]),
                (1, lambda jj: heads[1][0][:, jj * KT:(jj + 1) * KT],
                    lambda jj: heads[1][1]),
            ], work_q)
            attend(s, [
                (2, lambda jj: (dup2[0:64, jj * KT:(jj + 1) * KT] if jj % 2 == 0
                                else qkt2[64:128, jj * KT:(jj + 1) * KT]),
                    lambda jj: (qkt2[0:64, :] if jj % 2 == 0 else dup2[64:128, :])),
            ], work_q)
            # leftovers (small s): emit before the next q_super needs them
            while work_q:
                work_q.pop(0)()

        def norm_block(s):
            # normalization chains for supertile s — overlap with attend(s+1)
            qw = slice(s * QS, (s + 1) * QS)
            for h in range(HPC):
                rsb = ppool.tile([128, QS], BF16, tag="pt", name=f"nr{h}_{s}")
                with nc.allow_low_precision(reason="bf16 recip"):
                    nc.vector.reciprocal(rsb[64:65, :], ot[h][64:65, qw])
                rp = ot_ps.tile([65, QS], F32, tag="ot", name=f"nrp{h}_{s}")
                nc.tensor.matmul(
                    rp[:], ones65[64:65, :], rsb[64:65, :],
                    start=True, stop=True,
                )
                with nc.allow_low_precision(reason="bf16 norm"):
                    nc.vector.tensor_tensor(
                        ot[h][:, qw], ot[h][:, qw], rp[:], op=ALU.mult,
                    )

        # software-pipelined: qkv(it+1) units interleaved into attend(it)
        if do_qkv:
            for u in qkv_units(0):
                u()
        for it in range(NQS):
            work_q = []
            if do_qkv and it + 1 < NQS:
                work_q = qkv_units(it + 1)
            if do_attn:
                attend_all(it, work_q)
                if k_pv and k_norm:
                    norm_block(it)
            else:
                while work_q:
                    work_q.pop(0)()

        main_scope.close()

        # ---------------- output projection ----------------
        op_ps = ctx.enter_context(tc.tile_pool(name="opps", bufs=4, space="PSUM"))
        opool = ctx.enter_context(tc.tile_pool(name="osb", bufs=2))

        for tcn in range(T // 128) if do_proj else ():
            tw = slice(tcn * 128, (tcn + 1) * 128)
            osb = opool.tile([128, C], BF16, tag="osb", name=f"osb{tcn}")
            for half in range(2):
                cw = slice(half * 384, (half + 1) * 384)
                ps = op_ps.tile([128, 384], F32, tag="op", name=f"op{tcn}_{half}")
                for h in range(HPC):
                    nc.tensor.matmul(
                        ps[:], ot[h][0:64, tw], wout_sb[h][:, cw],
                        start=(h == 0), stop=(h == HPC - 1),
                    )
                with nc.allow_low_precision(reason="bf16 out"):
                    nc.vector.tensor_copy(osb[:, cw], ps[:])
            nc.sync.dma_start(out_p[tw, :], osb[:])
        if not do_proj:
            nc.sync.dma_start(out_p[0:128, :], qt01[:, 0:C])

        rep_scope.close()

    nc.compile()
    return nc


_NC_CACHE = {}


def _get_nc(with_qkv_bias: bool, repeat: int = 1, parts=('qkv', 'attn', 'proj')):
    key = (with_qkv_bias, repeat, tuple(parts))
    if key not in _NC_CACHE:
        _NC_CACHE[key] = build_nc(with_qkv_bias, repeat, parts)
    return _NC_CACHE[key]


def _prep_inputs(x, Wqkv, bqkv, Wout, bout):
    """Build the 8 per-core input maps (bf16 host-side casts)."""
    BF = mybir.dt.np(BF16)
    x = np.asarray(x, dtype=np.float32)
    Wqkv = np.asarray(Wqkv, dtype=np.float32)
    bqkv = np.asarray(bqkv, dtype=np.float32)
    Wout = np.asarray(Wout, dtype=np.float32)

    with_qkv_bias = bool(np.any(bqkv))

    # triangular block mask: tri[kk, qq] = qq >= kk
    kk = np.arange(128)[:, None]
    qq = np.arange(128)[None, :]
    tri = (qq >= kk).astype(np.float32)

    # [128, NCH, T]: partition-major chunked layout for one batched DMA/tile
    xts = [
        np.ascontiguousarray(
            x[b].T.reshape(NCH, 128, T).transpose(1, 0, 2)
        ).astype(BF)
        for b in range(B)
    ]

    in_maps = []
    for core in range(NCORES):
        b = core // GPB
        hs = [(core % GPB) * HPC + i for i in range(HPC)]  # 3 head indices

        def col(i, h):  # Wqkv column block for (q/k/v i, head h)
            return Wqkv[:, i * C + h * D: i * C + (h + 1) * D]

        def bias(i, h):
            return bqkv[i * C + h * D: i * C + (h + 1) * D]

        # packs: [q0|q1], [k0|k1], [q2|k2]
        wqk = np.concatenate(
            [col(0, hs[0]), col(0, hs[1]),
             col(1, hs[0]), col(1, hs[1]),
             col(0, hs[2]), col(1, hs[2])], axis=1,
        )  # [768, 384]
        wqk_c = wqk.reshape(NCH, 128, 384).transpose(1, 0, 2).reshape(128, NCH * 384)
        wqk_b = np.concatenate(
            [bias(0, hs[0]), bias(0, hs[1]),
             bias(1, hs[0]), bias(1, hs[1]),
             bias(0, hs[2]), bias(1, hs[2])]
        ).reshape(1, 384)

        # V: blocks of 65 cols per head [v_h | 1], padded to 256
        wv = np.zeros((C, 256), dtype=np.float32)
        wv_b = np.zeros((1, 256), dtype=np.float32)
        for i, h in enumerate(hs):
            wv[:, i * 65: i * 65 + 64] = col(2, h)
            wv_b[0, i * 65: i * 65 + 64] = bias(2, h)
            wv_b[0, i * 65 + 64] = 1.0
        if not with_qkv_bias:
            # ones column comes from the weights themselves: x @ wv has no
            # ones col, so fold it via an extra x-independent path is
            # unavailable — keep bias-matmul ones only when bias enabled.
            pass
        wv_c = wv.reshape(NCH, 128, 256).transpose(1, 0, 2).reshape(128, NCH * 256)

        wout_c = np.stack([Wout[h * D:(h + 1) * D, :] for h in hs])  # [3, 64, 768]

        in_maps.append({
            "xt": np.ascontiguousarray(xts[b]),
            "wqk": np.ascontiguousarray(wqk_c).astype(BF),
            "wqk_b": np.ascontiguousarray(wqk_b).astype(BF),
            "wv": np.ascontiguousarray(wv_c).astype(BF),
            "wv_b": np.ascontiguousarray(wv_b).astype(BF),
            "wout": np.ascontiguousarray(wout_c).astype(BF),
            "trid": tri.astype(BF),
            "ones_d": np.ones((1, QS), dtype=np.float32).astype(BF),
        })
    return in_maps, with_qkv_bias


def kernel(x, Wqkv, bqkv, Wout, bout, _trace=False, _trace_kwargs=None, _repeat=1,
           _parts=('qkv', 'attn', 'proj')):
    in_maps, with_qkv_bias = _prep_inputs(x, Wqkv, bqkv, Wout, bout)
    nc = _get_nc(with_qkv_bias, _repeat, _parts)
    res = run_bass_kernel_spmd(
        nc, in_maps, list(range(NCORES)), trace=_trace,
        **(_trace_kwargs or {}),
    )
    bout = np.asarray(bout, dtype=np.float32)
    parts = np.stack([res.results[i]["out_p"].astype(np.float32)
                      for i in range(NCORES)])
    out = parts.reshape(B, GPB, T, C).sum(axis=1) + bout
    kernel._last_result = res
    return out.astype(np.float32)


# revision 16
# speedup vs baseline: 1.1048x; 1.1048x over previous
"""Causal self-attention Trainium2 kernel (B=2, T=4096, C=768, H=12, D=64).

Sharding: 8 cores = 2 batches x 4 head-groups (3 heads each).
Each core computes, for its (batch b, heads h0..h2):
  - QKV projection from x[b].T (transposed + bf16-cast on host)
  - causal flash attention in score-transposed layout (S^T tiles [k=128, q=512])
  - output projection partial out_p = sum_h (O_h / l_h) @ Wout[h*64:(h+1)*64]
Host gathers: out[b] = sum of the 4 partials + bout.

v2: all matmul operands bf16 (fp32 PSUM accumulate); causal diagonal
supertiles are column-sliced so fully-masked q-subtiles are never computed,
exp'd, or PV'd; masking reduces to one [128,128] triangular block per
diagonal tile.
"""

import numpy as np
from contextlib import ExitStack

import concourse.bass as bass
import concourse.bacc as bacc
import concourse.mybir as mybir
import concourse.tile as tile
from concourse.bass_utils import run_bass_kernel_spmd

B, T, C, H, D = 2, 4096, 768, 12, 64
NCORES = 8
HPC = 3  # heads per core
GPB = 4  # head-groups per batch
SCALE = float(np.sqrt(D))  # 8.0
QS = 512  # q supertile (columns of S^T tiles)
KT = 128  # k tile (partitions of S^T tiles)
NQS = T // QS  # 8
NCH = C // 128  # 6 contraction chunks
G = 2  # S^T tiles per exp batch
VW = HPC * 64 + HPC  # vo block: [V0|1|V1|1|V2|1] = 195 cols

F32 = mybir.dt.float32
BF16 = mybir.dt.bfloat16
AX = mybir.AxisListType
ALU = mybir.AluOpType
ACTF = mybir.ActivationFunctionType


def build_nc(with_qkv_bias: bool, repeat: int = 1, parts=('qkv', 'attn', 'proj')):
    nc = bacc.Bacc()

    xt = nc.dram_tensor("xt", [C, T], BF16, kind="ExternalInput")
    wqk = nc.dram_tensor("wqk", [128, NCH * 384], BF16, kind="ExternalInput")
    wqk_b = nc.dram_tensor("wqk_b", [1, 384], BF16, kind="ExternalInput")
    wv = nc.dram_tensor("wv", [128, NCH * 256], BF16, kind="ExternalInput")
    wv_b = nc.dram_tensor("wv_b", [1, 256], BF16, kind="ExternalInput")
    wout = nc.dram_tensor("wout", [HPC, 64, C], BF16, kind="ExternalInput")
    trid = nc.dram_tensor("trid", [128, 128], BF16, kind="ExternalInput")
    ones_d = nc.dram_tensor("ones_d", [1, QS], BF16, kind="ExternalInput")
    out_p = nc.dram_tensor("out_p", [T, C], BF16, kind="ExternalOutput")

    do_qkv = 'qkv' in parts
    do_attn = 'attn' in parts
    do_proj = 'proj' in parts
    k_exp = 'noexp' not in parts
    k_mask = 'nomask' not in parts
    k_pv = 'nopv' not in parts
    k_norm = 'nonorm' not in parts

    with tile.TileContext(nc) as tc, ExitStack() as ctx:
        rep_scope = ExitStack()
        if repeat > 1:
            rep_scope.enter_context(tc.For_i(0, repeat, 1))
        const = ctx.enter_context(tc.tile_pool(name="const", bufs=1))

        wqk_sb = const.tile([128, NCH * 384], BF16, tag="wqk")
        nc.sync.dma_start(wqk_sb[:], wqk[:])
        wv_sb = const.tile([128, NCH * 256], BF16, tag="wv")
        nc.sync.dma_start(wv_sb[:], wv[:])
        wvb_sb = const.tile([1, 256], BF16, tag="wvb")
        # wv_b always carries the ones column for the softmax denominator
        nc.sync.dma_start(wvb_sb[:], wv_b[:])
        wqkb_sb = const.tile([1, 384], BF16, tag="wqkb")
        if with_qkv_bias:
            nc.sync.dma_start(wqkb_sb[:], wqk_b[:])
        tri_sb = const.tile([128, 128], BF16, tag="tri")
        nc.sync.dma_start(tri_sb[:], trid[:])
        wout_sb = []
        for h in range(HPC):
            wsb = const.tile([64, C], BF16, tag=f"wout{h}", name=f"wout_sb{h}")
            nc.sync.dma_start(wsb[:], wout[h])
            wout_sb.append(wsb)

        ones_row = const.tile([1, QS], BF16, tag="ones_row")
        nc.sync.dma_start(ones_row[:], ones_d[:])
        ones65 = const.tile([128, 65], BF16, tag="ones65")
        nc.sync.dma_start(ones65[64:65, :], ones_d[:, 0:65])

        qt01 = const.tile([128, T], BF16, tag="qt01")
        kt01 = const.tile([128, T], BF16, tag="kt01")
        qkt2 = const.tile([128, T], BF16, tag="qkt2")
        dup2 = const.tile([128, T], BF16, tag="dup2")
        vo = const.tile([128, (T // 128) * VW], BF16, tag="vo")
        ot = [const.tile([65, T], BF16, tag=f"ot{h}", name=f"ot{h}") for h in range(HPC)]

        main_scope = ExitStack()
        xpool = main_scope.enter_context(tc.tile_pool(name="xt", bufs=3))
        qkv_ps = main_scope.enter_context(tc.tile_pool(name="qkvps", bufs=2, space="PSUM"))
        sg_ps = main_scope.enter_context(tc.tile_pool(name="sgps", bufs=2, space="PSUM"))
        ot_ps = main_scope.enter_context(tc.tile_pool(name="otps", bufs=2, space="PSUM"))
        ppool = main_scope.enter_context(tc.tile_pool(name="pt", bufs=4))

        def qkv_units(it):
            """Emit x DMAs now; return PE work-unit closures to interleave."""
            tw = slice(it * QS, (it + 1) * QS)
            xts = []
            for c in range(NCH):
                xtile = xpool.tile([128, QS], BF16, tag=f"x{c % 3}", name=f"xt{it}_{c}")
                nc.sync.dma_start(xtile[:], xt[c * 128:(c + 1) * 128, tw])
                xts.append(xtile)
            units = []

            def pack_unit(p, dest):
                def emit():
                    ps = qkv_ps.tile([128, QS], F32, tag="qkv", name=f"qk{it}_{p}")
                    for c in range(NCH):
                        nc.tensor.matmul(
                            ps[:],
                            wqk_sb[:, c * 384 + p * 128: c * 384 + (p + 1) * 128],
                            xts[c][:],
                            start=(c == 0),
                            stop=(not with_qkv_bias and c == NCH - 1),
                        )
                    if with_qkv_bias:
                        nc.tensor.matmul(
                            ps[:], wqkb_sb[:, p * 128:(p + 1) * 128], ones_row[:],
                            start=False, stop=True,
                        )
                    with nc.allow_low_precision(reason="bf16 matmul operand"):
                        nc.vector.tensor_copy(dest[:, tw], ps[:])
                    if p == 2:
                        nc.sync.dma_start(dup2[0:64, tw], qkt2[64:128, tw])
                        nc.sync.dma_start(dup2[64:128, tw], qkt2[0:64, tw])
                return emit

            def v_unit(st):
                def emit():
                    tcn = it * (QS // 128) + st
                    ps = qkv_ps.tile([128, 256], F32, tag="qkv", name=f"v{it}_{st}")
                    for c in range(NCH):
                        nc.tensor.matmul(
                            ps[:],
                            xts[c][:, st * 128:(st + 1) * 128],
                            wv_sb[:, c * 256:(c + 1) * 256],
                            start=(c == 0),
                            stop=False,
                        )
                    # always: injects the ones column (softmax denominator)
                    nc.tensor.matmul(
                        ps[:], ones_row[:, st * 128:(st + 1) * 128], wvb_sb[:],
                        start=False, stop=True,
                    )
                    with nc.allow_low_precision(reason="bf16 matmul operand"):
                        nc.vector.tensor_copy(
                            vo[:, tcn * VW:(tcn + 1) * VW], ps[:, 0:VW]
                        )
                return emit

            for p, dest in ((0, qt01), (1, kt01), (2, qkt2)):
                units.append(pack_unit(p, dest))
            for st in range(QS // 128):
                units.append(v_unit(st))
            return units

        def attend(s, jobs, work_q=None):
            """jobs: list of (h, ktb_fn, qtb_fn)."""
            qw = slice(s * QS, (s + 1) * QS)
            nk = 4 * (s + 1)
            otps = {}
            for h, _, _ in jobs:
                otps[h] = ot_ps.tile([65, QS], F32, tag="ot", name=f"otp{h}_{s}")

            def mstart(jj):  # first unmasked col (within the 512-q window)
                m = jj - (nk - 4)
                return m * 128 if m > 0 else 0

            def emit_pv(pts, g0, gn):
                for h, _, _ in jobs:
                    for jl in range(gn):
                        jj = g0 + jl
                        c0 = mstart(jj)
                        m = jj - (nk - 4)
                        if k_mask and m >= 0:
                            # triangle block only: q-subtile == k-tile index
                            blk = slice(jl * QS + c0, jl * QS + c0 + 128)
                            nc.vector.tensor_tensor(
                                pts[h][:, blk], pts[h][:, blk], tri_sb[:],
                                op=ALU.mult,
                            )
                        if k_pv:
                            nc.tensor.matmul(
                                otps[h][:, c0:QS],
                                vo[:, jj * VW + h * 65: jj * VW + (h + 1) * 65],
                                pts[h][:, jl * QS + c0:(jl + 1) * QS],
                                start=(jj == 0),
                                stop=(jj == nk - 1),
                                skip_group_check=True,
                            )

            prev = None
            for g0 in range(0, nk, G):
                gn = min(G, nk - g0)
                sgps, pts = {}, {}
                for h, ktb_fn, qtb_fn in jobs:
                    sgps[h] = sg_ps.tile([128, G * QS], F32, tag="sg",
                                         name=f"sg{h}_{s}_{g0}")
                for jl in range(gn):
                    jj = g0 + jl
                    c0 = mstart(jj)
                    for h, ktb_fn, qtb_fn in jobs:
                        nc.tensor.matmul(
                            sgps[h][:, jl * QS + c0:(jl + 1) * QS],
                            ktb_fn(jj),
                            qtb_fn(jj)[:, s * QS + c0:(s + 1) * QS],
                            start=True, stop=True,
                        )
                # PV for the previous group fills PE while ACT exps this group
                if prev is not None:
                    emit_pv(*prev)
                if work_q:
                    work_q.pop(0)()
                for h, _, _ in jobs:
                    pt = ppool.tile([128, G * QS], BF16, tag="pt",
                                    name=f"pt{h}_{s}_{g0}")
                    if k_exp:
                        # fuse contiguous full tiles into one ACT instr
                        runs = []
                        for jl in range(gn):
                            c0 = mstart(g0 + jl)
                            lo, hi = jl * QS + c0, (jl + 1) * QS
                            if runs and runs[-1][1] == lo and c0 == 0:
                                runs[-1][1] = hi
                            else:
                                runs.append([lo, hi])
                        for lo, hi in runs:
                            nc.scalar.activation(
                                pt[:, lo:hi], sgps[h][:, lo:hi], ACTF.Exp,
                                scale=1.0 / SCALE,
                            )
                    pts[h] = pt
                prev = (pts, g0, gn)
            if prev is not None:
                emit_pv(*prev)
            for h, _, _ in jobs:
                if not k_pv:
                    continue
                with nc.allow_low_precision(reason="bf16 matmul operand"):
                    nc.vector.tensor_copy(ot[h][:, qw], otps[h][:])

        heads = (
            (kt01[0:64, :], qt01[0:64, :]),
            (kt01[64:128, :], qt01[64:128, :]),
        )

        def attend_all(s, work_q):
            attend(s, [
                (0, lambda jj: heads[0][0][:, jj * KT:(jj + 1) * KT],
                    lambda jj: heads[0][1]),
                (1, lambda jj: heads[1][0][:, jj * KT:(jj + 1) * KT],
                    lambda jj: heads[1][1]),
            ], work_q)
            attend(s, [
                (2, lambda jj: (dup2[0:64, jj * KT:(jj + 1) * KT] if jj % 2 == 0
                                else qkt2[64:128, jj * KT:(jj + 1) * KT]),
                    lambda jj: (qkt2[0:64, :] if jj % 2 == 0 else dup2[64:128, :])),
            ], work_q)
            # leftovers (small s): emit before the next q_super needs them
            while work_q:
                work_q.pop(0)()

        # software-pipelined: qkv(it+1) units interleaved into attend(it)
        if do_qkv:
            for u in qkv_units(0):
                u()
        for it in range(NQS):
            work_q = []
            if do_qkv and it + 1 < NQS:
                work_q = qkv_units(it + 1)
            if do_attn:
                attend_all(it, work_q)
            else:
                while work_q:
                    work_q.pop(0)()

        # deferred normalization: independent chains, pipelined
        if do_attn and k_pv and k_norm:
            for h in range(HPC):
                for s in range(NQS):
                    qw = slice(s * QS, (s + 1) * QS)
                    rsb = ppool.tile([128, QS], BF16, tag="pt", name=f"nr{h}_{s}")
                    with nc.allow_low_precision(reason="bf16 recip"):
                        nc.vector.reciprocal(rsb[64:65, :], ot[h][64:65, qw])
                    rp = ot_ps.tile([65, QS], F32, tag="ot", name=f"nrp{h}_{s}")
                    nc.tensor.matmul(
                        rp[:], ones65[64:65, :], rsb[64:65, :],
                        start=True, stop=True,
                    )
                    with nc.allow_low_precision(reason="bf16 norm"):
                        nc.vector.tensor_tensor(
                            ot[h][:, qw], ot[h][:, qw], rp[:], op=ALU.mult,
                        )

        main_scope.close()

        # ---------------- output projection ----------------
        op_ps = ctx.enter_context(tc.tile_pool(name="opps", bufs=4, space="PSUM"))
        opool = ctx.enter_context(tc.tile_pool(name="osb", bufs=2))

        for tcn in range(T // 128) if do_proj else ():
            tw = slice(tcn * 128, (tcn + 1) * 128)
            osb = opool.tile([128, C], BF16, tag="osb", name=f"osb{tcn}")
            for half in range(2):
                cw = slice(half * 384, (half + 1) * 384)
                ps = op_ps.tile([128, 384], F32, tag="op", name=f"op{tcn}_{half}")
                for h in range(HPC):
                    nc.tensor.matmul(
                        ps[:], ot[h][0:64, tw], wout_sb[h][:, cw],
                        start=(h == 0), stop=(h == HPC - 1),
                    )
                with nc.allow_low_precision(reason="bf16 out"):
                    nc.vector.tensor_copy(osb[:, cw], ps[:])
            nc.sync.dma_start(out_p[tw, :], osb[:])
        if not do_proj:
            nc.sync.dma_start(out_p[0:128, :], qt01[:, 0:C])

        rep_scope.close()

    nc.compile()
    return nc


_NC_CACHE = {}


def _get_nc(with_qkv_bias: bool, repeat: int = 1, parts=('qkv', 'attn', 'proj')):
    key = (with_qkv_bias, repeat, tuple(parts))
    if key not in _NC_CACHE:
        _NC_CACHE[key] = build_nc(with_qkv_bias, repeat, parts)
    return _NC_CACHE[key]


def _prep_inputs(x, Wqkv, bqkv, Wout, bout):
    """Build the 8 per-core input maps (bf16 host-side casts)."""
    BF = mybir.dt.np(BF16)
    x = np.asarray(x, dtype=np.float32)
    Wqkv = np.asarray(Wqkv, dtype=np.float32)
    bqkv = np.asarray(bqkv, dtype=np.float32)
    Wout = np.asarray(Wout, dtype=np.float32)

    with_qkv_bias = bool(np.any(bqkv))

    # triangular block mask: tri[kk, qq] = qq >= kk
    kk = np.arange(128)[:, None]
    qq = np.arange(128)[None, :]
    tri = (qq >= kk).astype(np.float32)

    xts = [np.ascontiguousarray(x[b].T).astype(BF) for b in range(B)]  # [C, T]

    in_maps = []
    for core in range(NCORES):
        b = core // GPB
        hs = [(core % GPB) * HPC + i for i in range(HPC)]  # 3 head indices

        def col(i, h):  # Wqkv column block for (q/k/v i, head h)
            return Wqkv[:, i * C + h * D: i * C + (h + 1) * D]

        def bias(i, h):
            return bqkv[i * C + h * D: i * C + (h + 1) * D]

        # packs: [q0|q1], [k0|k1], [q2|k2]
        wqk = np.concatenate(
            [col(0, hs[0]), col(0, hs[1]),
             col(1, hs[0]), col(1, hs[1]),
             col(0, hs[2]), col(1, hs[2])], axis=1,
        )  # [768, 384]
        wqk_c = wqk.reshape(NCH, 128, 384).transpose(1, 0, 2).reshape(128, NCH * 384)
        wqk_b = np.concatenate(
            [bias(0, hs[0]), bias(0, hs[1]),
             bias(1, hs[0]), bias(1, hs[1]),
             bias(0, hs[2]), bias(1, hs[2])]
        ).reshape(1, 384)

        # V: blocks of 65 cols per head [v_h | 1], padded to 256
        wv = np.zeros((C, 256), dtype=np.float32)
        wv_b = np.zeros((1, 256), dtype=np.float32)
        for i, h in enumerate(hs):
            wv[:, i * 65: i * 65 + 64] = col(2, h)
            wv_b[0, i * 65: i * 65 + 64] = bias(2, h)
            wv_b[0, i * 65 + 64] = 1.0
        if not with_qkv_bias:
            # ones column comes from the weights themselves: x @ wv has no
            # ones col, so fold it via an extra x-independent path is
            # unavailable — keep bias-matmul ones only when bias enabled.
            pass
        wv_c = wv.reshape(NCH, 128, 256).transpose(1, 0, 2).reshape(128, NCH * 256)

        wout_c = np.stack([Wout[h * D:(h + 1) * D, :] for h in hs])  # [3, 64, 768]

        in_maps.append({
            "xt": np.ascontiguousarray(xts[b]),
            "wqk": np.ascontiguousarray(wqk_c).astype(BF),
            "wqk_b": np.ascontiguousarray(wqk_b).astype(BF),
            "wv": np.ascontiguousarray(wv_c).astype(BF),
            "wv_b": np.ascontiguousarray(wv_b).astype(BF),
            "wout": np.ascontiguousarray(wout_c).astype(BF),
            "trid": tri.astype(BF),
            "ones_d": np.ones((1, QS), dtype=np.float32).astype(BF),
        })
    return in_maps, with_qkv_bias


def kernel(x, Wqkv, bqkv, Wout, bout, _trace=False, _trace_kwargs=None, _repeat=1,
           _parts=('qkv', 'attn', 'proj')):
    in_maps, with_qkv_bias = _prep_inputs(x, Wqkv, bqkv, Wout, bout)
    nc = _get_nc(with_qkv_bias, _repeat, _parts)
    res = run_bass_kernel_spmd(
        nc, in_maps, list(range(NCORES)), trace=_trace,
        **(_trace_kwargs or {}),
    )
    bout = np.asarray(bout, dtype=np.float32)
    parts = np.stack([res.results[i]["out_p"].astype(np.float32)
                      for i in range(NCORES)])
    out = parts.reshape(B, GPB, T, C).sum(axis=1) + bout
    kernel._last_result = res
    return out.astype(np.float32)
